# revision 1
# baseline (speedup 1.0000x reference)
"""Trainium2 Bass kernel for a DiT-style transformer block (adaLN modulation,
RoPE self-attention with additive rank mask, hybrid cross-attention to
[clean|observed] memory, gated MLP).

Sharding: 8 cores = 4 batches x 2 sequence-halves. Each core computes the
block output for its 512 query tokens of one batch. Per-core token order is
permuted (host side) so the core's own tokens come first, which keeps the
program SPMD-static across cores.

Layout: activations live feature-major ("T-layout", [feature, token]) so all
matmuls contract along partitions. Matmul operands use dtype float32r
(full-rate PE, ~1.5e-4 rms rel error vs fp32). Softmax runs without
max-subtraction (scores are O(10)); masking multiplies probabilities by
exp(mask) in {0,1}. Softmax denominators come free from a ones-column
appended to each head's value block (p@v output row 64). The memory layernorm
is folded through the KV projection (per-token affine commutes with the
feature contraction): kv = rs_t*(W@mem) - (mu*rs)_t*rowsum(W).
"""

import numpy as np
from contextlib import ExitStack

from concourse import bacc, mybir
import concourse.bass as bass
import concourse.tile as tile
from concourse import bass_utils

F32 = mybir.dt.float32
F32R = mybir.dt.float32r
AF = mybir.ActivationFunctionType
OP = mybir.AluOpType

P = 128


class Cfg:
    def __init__(self, mini=False):
        if mini:
            self.B, self.N, self.D, self.H, self.HD = 2, 256, 256, 4, 64
            self.COND = 128
        else:
            self.B, self.N, self.D, self.H, self.HD = 4, 1024, 1024, 16, 64
            self.COND = 256
        self.DH = 4 * self.D
        self.SQ = self.N // 2            # own query tokens per core
        self.CH = self.D // P            # d-chunks
        self.HH = self.H * self.HD // P  # head-pair chunks (= H // 2)
        self.KK = self.N // P            # key chunks per N tokens
        self.NF = self.N // self.SQ      # token-free blocks of SQ (=2)
        self.CC = self.COND // P
        self.DHC = self.DH // P
        self.QKK = self.SQ // P          # key chunks per memory quarter
        self.n_cores = 2 * self.B
        self.eps = 1e-5


def _dma_bcast(nc, out_tile, dram_ap, off, n):
    """DMA dram row [1, off:off+n] broadcast to all partitions [P, n]."""
    src = bass.AP(
        tensor=dram_ap.tensor, offset=dram_ap.offset + off, ap=[[0, P], [1, n]]
    )
    nc.gpsimd.dma_start(out=out_tile, in_=src)


def _shift32_dma(nc, dst, src):
    """dst[p] = src[p xor-32 within each 64-block], [128, F] SBUF tiles."""
    for blk in range(2):
        b = blk * 64
        nc.sync.dma_start(out=dst[b : b + 32, :], in_=src[b + 32 : b + 64, :])
        nc.sync.dma_start(out=dst[b + 32 : b + 64, :], in_=src[b : b + 32, :])


def r(ap):
    """fp32 view of an f32r AP for DVE/ACT/gpsimd input reads."""
    return ap.bitcast(F32)


def build_program(cfg: Cfg):
    c = cfg
    nc = bacc.Bacc(
        "TRN2",
        target_bir_lowering=False,
        debug=False,
        enable_asserts=True,
        num_devices=c.n_cores,
    )

    def din(name, shape, dt=F32R):
        return nc.dram_tensor(name, shape, dt, kind="ExternalInput").ap()

    xT = din("xT", [c.D, c.N])
    tcT = din("tcT", [c.COND, c.N])
    hcT = din("hcT", [c.D, c.N])
    hoT = din("hoT", [c.D, c.N])
    wadaT = din("wadaT", [c.COND, 9 * c.D])
    wqkvT = din("wqkvT", [c.D, 3 * c.D])
    wselfT = din("wselfT", [c.D, c.D])
    wqT = din("wqT", [c.D, c.D])
    wkvT = din("wkvT", [c.D, 2 * c.D])
    wcrossT = din("wcrossT", [c.D, c.D])
    wm1T = din("wm1T", [c.D, c.DH])
    wm2T = din("wm2T", [c.DH, c.D])
    bada = din("bada", [P, 9 * c.CH], F32)
    bm1 = din("bm1", [P, c.DHC], F32)
    bm2 = din("bm2", [P, c.CH], F32)
    cqt = din("cqt", [P, c.SQ], F32)
    sqt = din("sqt", [P, c.SQ], F32)
    ckts = din("ckts", [P, c.N], F32)
    skts = din("skts", [P, c.N], F32)
    cktm = din("cktm", [P, c.N], F32)
    sktm = din("sktm", [P, c.N], F32)
    mself = din("mself", [c.N, c.SQ], F32)
    mhc = din("mhc", [c.N, c.SQ], F32)
    mho = din("mho", [c.N, c.SQ], F32)
    la_self = din("la_self", [1, c.N], F32)   # rstd per own-order token
    lb_self = din("lb_self", [1, c.N], F32)   # mean per own-order token
    la_mc = din("la_mc", [1, c.N], F32)       # rstd, clean memory
    lb_mc = din("lb_mc", [1, c.N], F32)       # mean*rstd, clean memory
    la_mo = din("la_mo", [1, c.N], F32)
    lb_mo = din("lb_mo", [1, c.N], F32)
    swk = din("swk", [P, c.HH], F32)          # rowsum(Wk) per k-feature
    wsumv = din("wsumv", [1, c.H * c.HD], F32)  # rowsum(Wv) per v-feature
    rs_cols = din("rs_cols", [P, 2 * c.KK], F32)    # mem rstd, column layout
    mrs_cols = din("mrs_cols", [P, 2 * c.KK], F32)  # mem mean*rstd, columns
    out_d = nc.dram_tensor("out", [c.D, c.SQ], F32, kind="ExternalOutput").ap()
    xc_d = nc.dram_tensor("xc_scratch", [c.D, c.SQ], F32R, kind="Internal").ap()
    xc2_d = nc.dram_tensor("xc2_scratch", [c.D, c.SQ], F32R, kind="Internal").ap()

    with ExitStack() as ctx:
        tc = ctx.enter_context(tile.TileContext(nc))
        persist = ctx.enter_context(tc.tile_pool(name="persist", bufs=1))
        ws = ctx.enter_context(tc.tile_pool(name="wstream", bufs=1))
        tw_pool = ctx.enter_context(tc.tile_pool(name="tw", bufs=6))
        rsp = ctx.enter_context(tc.tile_pool(name="rsp", bufs=1))
        small = ctx.enter_context(tc.tile_pool(name="small", bufs=1))

        def wtile():
            return ws.tile([P, P], F32R, name="wt", tag="wt", bufs=8)

        def wbtile(nk):
            return ws.tile([P, nk, P], F32R, name=f"wb{nk}", tag=f"wb{nk}",
                           bufs=3)


        def tw():
            return tw_pool.tile([P, c.SQ], F32, name="tw", tag="tw")

        # ---------- persistent preloads ----------
        TC = persist.tile([P, c.CC, c.N], F32R)
        nc.sync.dma_start(out=TC, in_=tcT.rearrange("(k p) n -> p k n", p=P))
        CQ = persist.tile([P, c.SQ], F32)
        nc.sync.dma_start(out=CQ, in_=cqt)
        SQt = persist.tile([P, c.SQ], F32)
        nc.sync.dma_start(out=SQt, in_=sqt)
        BADA = persist.tile([P, 9 * c.CH], F32)
        nc.sync.dma_start(out=BADA, in_=bada)
        BM1 = persist.tile([P, c.DHC], F32)
        nc.sync.dma_start(out=BM1, in_=bm1)
        BM2 = persist.tile([P, c.CH], F32)
        nc.sync.dma_start(out=BM2, in_=bm2)

        EPS = persist.tile([P, 1], F32)
        nc.vector.memset(EPS, 1e-5)
        ones_f32 = persist.tile([P, 16], F32)
        nc.vector.memset(ones_f32, 1.0)
        ONE = persist.tile([P, 1], F32R)
        nc.vector.tensor_copy(ONE, ones_f32[:, 0:1])
        ONES16 = persist.tile([P, 16], F32R)
        nc.vector.tensor_copy(ONES16, ones_f32)

        # ---------- helpers ----------
        def ada_modulate(q_sh, q_sc, x_src, x_nf, la_b, lb_b, xn_dst):
            """xn = x*sc1 - m*sc1 + sh, with sc1 = rs*w*(1+sc).

            la_b(cols) -> [P, SQ] rstd broadcast AP; lb_b(cols) -> mean.
            x_src(j, tf) / xn_dst(j, tf): [P, SQ] APs.
            """
            with tc.tile_pool(name="ps_ada", bufs=1, space="PSUM") as psa:
                for j in range(c.CH):
                    ps_sh = [
                        psa.tile([P, c.SQ], F32, name=f"ps_sh{t}", tag=f"ps_sh{t}")
                        for t in range(x_nf)
                    ]
                    ps_sc = [
                        psa.tile([P, c.SQ], F32, name=f"ps_sc{t}", tag=f"ps_sc{t}")
                        for t in range(x_nf)
                    ]
                    wt = wbtile(c.CC)
                    nc.sync.dma_start(
                        out=wt,
                        in_=wadaT[
                            :, q_sh * c.D + j * P : q_sh * c.D + (j + 1) * P
                        ].rearrange("(k p) m -> p k m", p=P),
                    )
                    wt2 = wbtile(c.CC)
                    nc.sync.dma_start(
                        out=wt2,
                        in_=wadaT[
                            :, q_sc * c.D + j * P : q_sc * c.D + (j + 1) * P
                        ].rearrange("(k p) m -> p k m", p=P),
                    )
                    for k in range(c.CC):
                        for tf in range(x_nf):
                            nc.tensor.matmul(
                                ps_sh[tf], wt[:, k, :],
                                TC[:, k, tf * c.SQ : (tf + 1) * c.SQ],
                                start=(k == 0), stop=(k == c.CC - 1),
                            )
                        for tf in range(x_nf):
                            nc.tensor.matmul(
                                ps_sc[tf], wt2[:, k, :],
                                TC[:, k, tf * c.SQ : (tf + 1) * c.SQ],
                                start=(k == 0), stop=(k == c.CC - 1),
                            )
                    for tf in range(x_nf):
                        cols = slice(tf * c.SQ, (tf + 1) * c.SQ)
                        sc1 = tw()
                        nc.vector.scalar_tensor_tensor(
                            out=sc1, in0=ps_sc[tf],
                            scalar=BADA[:, q_sc * c.CH + j : q_sc * c.CH + j + 1],
                            in1=la_b(cols), op0=OP.add, op1=OP.mult,
                        )
                        mm = tw()
                        nc.vector.tensor_mul(mm, lb_b(cols), sc1)
                        sh = tw()
                        nc.vector.scalar_tensor_tensor(
                            out=sh, in0=ps_sh[tf],
                            scalar=BADA[:, q_sh * c.CH + j : q_sh * c.CH + j + 1],
                            in1=mm, op0=OP.add, op1=OP.subtract,
                        )
                        t = tw()
                        nc.vector.tensor_mul(t, x_src(j, tf), sc1)
                        nc.vector.tensor_add(xn_dst(j, tf), t, sh)

        def ada_gate_one(q_g, j, psg):
            """Return a [P, SQ] f32 tile holding gate chunk j on demand."""
            ps = psg.tile([P, c.SQ], F32, name="ps_g", tag="ps_g")
            wt = wbtile(c.CC)
            nc.sync.dma_start(
                out=wt,
                in_=wadaT[
                    :, q_g * c.D + j * P : q_g * c.D + (j + 1) * P
                ].rearrange("(k p) m -> p k m", p=P),
            )
            for k in range(c.CC):
                nc.tensor.matmul(
                    ps, wt[:, k, :], TC[:, k, 0 : c.SQ],
                    start=(k == 0), stop=(k == c.CC - 1),
                )
            g = tw()
            nc.vector.tensor_scalar_add(
                g, ps, BADA[:, q_g * c.CH + j : q_g * c.CH + j + 1]
            )
            return g

        def rope_evict(zsrc, hh, cols_t, ctab, stab, dst):
            """dst[:, hh, cols_t] = zsrc*cos + shift32(zsrc)*sin_signed."""
            t1 = tw()
            nc.vector.tensor_mul(t1, zsrc, ctab)
            tsh = tw()
            _shift32_dma(nc, tsh, zsrc)
            nc.vector.tensor_mul(tsh, tsh, stab)
            nc.vector.tensor_add(dst[:, hh, cols_t], t1, tsh)

        def proj_rope(wT_dram, col_off, n_free, ctab, stab, dst, src_tile):
            """dst[:, hh, :] = rope(W[:, cols].T @ src), head-pair chunks."""
            nf = n_free // c.SQ
            with tc.tile_pool(name="ps_qk", bufs=3, space="PSUM") as psq:
                for hh in range(c.HH):
                    wt = wbtile(c.CH)
                    nc.sync.dma_start(
                        out=wt,
                        in_=wT_dram[
                            :, col_off + hh * P : col_off + (hh + 1) * P
                        ].rearrange("(k p) m -> p k m", p=P),
                    )
                    for tf in range(nf):
                        ps = psq.tile([P, c.SQ], F32, name="ps_qk", tag="ps_qk")
                        for k in range(c.CH):
                            nc.tensor.matmul(
                                ps, wt[:, k, :],
                                src_tile[:, k, tf * c.SQ : (tf + 1) * c.SQ],
                                start=(k == 0), stop=(k == c.CH - 1),
                            )
                        cols = slice(tf * c.SQ, (tf + 1) * c.SQ)
                        traw = tw()
                        nc.scalar.activation(traw, ps, AF.Copy)
                        rope_evict(
                            traw, hh, cols, ctab[:, cols], stab[:, cols], dst
                        )

        def vproj_self(src_tile, vdst, wvp):
            """Token-major value projection from resident XN; ones cols."""
            ntt = c.KK
            ffw = min(512, c.H * c.HD)
            nff = (c.H * c.HD) // ffw
            hpf = ffw // 64
            for tt in range(ntt):
                ap = vdst[:, tt, :].rearrange("p (h e) -> p h e", e=65)[:, :, 64:65]
                nc.vector.tensor_copy(ap, ONES16[:, 0 : c.H])
            with tc.tile_pool(name="ps_v", bufs=8, space="PSUM") as psv:
                for ff in range(nff):
                    pss = [
                        psv.tile([P, ffw], F32, name="ps_v", tag="ps_v")
                        for _ in range(ntt)
                    ]
                    kh = max(1, c.CH // 4)
                    for kg in range(c.CH // kh):
                        wt = wvp.tile([P, kh, ffw], F32R, name="wv", tag="wv",
                                      bufs=2)
                        nc.sync.dma_start(
                            out=wt,
                            in_=wqkvT[
                                kg * kh * P : (kg + 1) * kh * P,
                                2 * c.D + ff * ffw : 2 * c.D + (ff + 1) * ffw,
                            ].rearrange("(k p) m -> p k m", p=P),
                        )
                        for k in range(kh):
                            gk = kg * kh + k
                            for tt in range(ntt):
                                nc.tensor.matmul(
                                    pss[tt],
                                    src_tile[:, gk, tt * P : (tt + 1) * P],
                                    wt[:, k, :],
                                    start=(gk == 0), stop=(gk == c.CH - 1),
                                )
                    for tt in range(ntt):
                        ap = (
                            vdst[:, tt, ff * hpf * 65 : (ff + 1) * hpf * 65]
                            .rearrange("p (h e) -> p h e", e=65)[:, :, 0:64]
                        )
                        nc.vector.tensor_copy(ap, pss[tt])

        def attention_hp(hp, khat, vtile, qhat, masks, n_kk, ps_o1, ps_o2,
                         tp_pool, first, last):
            """One head pair, software-pipelined: p@v lags scores by one
            chunk so the PE has independent work while ACT/DVE/GpSimd chew
            through exp+mask of the current chunk."""
            h1, h2 = 2 * hp, 2 * hp + 1

            def pv(kkc, pt):
                nc.tensor.matmul(
                    ps_o1, vtile[:, kkc, h1 * 65 : (h1 + 1) * 65],
                    pt[:, 0 : c.SQ],
                    start=(first and kkc == 0),
                    stop=(last and kkc == n_kk - 1),
                )
                nc.tensor.matmul(
                    ps_o2, vtile[:, kkc, h2 * 65 : (h2 + 1) * 65],
                    pt[:, c.SQ : 2 * c.SQ],
                    start=(first and kkc == 0),
                    stop=(last and kkc == n_kk - 1),
                )

            with tc.tile_pool(name="ps_s", bufs=2, space="PSUM") as pss:
                prev = None
                for kkc in range(n_kk):
                    ps = pss.tile([P, 2 * c.SQ], F32, name="ps_s", tag="ps_s")
                    ks = slice(kkc * P, (kkc + 1) * P)
                    nc.tensor.matmul(
                        ps[:, 0 : c.SQ],
                        khat[0:64, hp, ks], qhat[0:64, hp, :],
                        start=True, stop=True,
                    )
                    nc.tensor.matmul(
                        ps[:, c.SQ : 2 * c.SQ],
                        khat[64:128, hp, ks], qhat[64:128, hp, :],
                        start=True, stop=True,
                    )
                    pt = tp_pool.tile(
                        [P, 2 * c.SQ], F32R, name="t_p", tag="t_p", bufs=3
                    )
                    nc.scalar.activation(pt, ps, AF.Exp)
                    nc.vector.tensor_mul(
                        pt[:, 0 : c.SQ], r(pt[:, 0 : c.SQ]), masks[:, kkc, :]
                    )
                    eng2 = nc.vector if (kkc % 3 == 2) else nc.gpsimd
                    eng2.tensor_mul(
                        pt[:, c.SQ : 2 * c.SQ], r(pt[:, c.SQ : 2 * c.SQ]),
                        masks[:, kkc, :],
                    )
                    if prev is not None:
                        pv(*prev)
                    prev = (kkc, pt)
                pv(*prev)

        def evict_unnorm(ps_o, hp, second, odst, den, tp_pool):
            """Stage unnormalized o rows into odst and the denominator row
            into den[2hp+second]. Normalization happens batched later."""
            h = 2 * hp + (1 if second else 0)
            dstage = tp_pool.tile(
                [65, c.SQ], F32, name="t_dstage", tag="t_dstage", bufs=2
            )
            nc.vector.tensor_copy(dstage[64:65, :], ps_o[64:65, :])
            nc.sync.dma_start(out=den[h : h + 1, :], in_=dstage[64:65, :])
            if not second:
                nc.vector.tensor_copy(odst[0:64, hp, :], ps_o[0:64, :])
            else:
                st = tp_pool.tile(
                    [64, c.SQ], F32R, name="t_onorm", tag="t_onorm", bufs=2
                )
                nc.vector.tensor_copy(st, ps_o[0:64, :])
                nc.sync.dma_start(out=odst[64:128, hp, :], in_=st)

        def normalize_batch(odst, den, deni, tp_pool, n_hp):
            """odst[:, hp, :] *= 1/den rows (broadcast per head)."""
            nc.vector.reciprocal(deni, den)
            for hp in range(n_hp):
                d1 = small.tile([1, c.SQ], F32, name="s_d1", tag="s_d1",
                                bufs=2)
                nc.sync.dma_start(out=d1, in_=deni[2 * hp : 2 * hp + 1, :])
                d2 = small.tile([1, c.SQ], F32, name="s_d2", tag="s_d2",
                                bufs=2)
                nc.sync.dma_start(out=d2, in_=deni[2 * hp + 1 : 2 * hp + 2, :])
                rb = tp_pool.tile(
                    [P, c.SQ], F32, name="t_rb", tag="t_rb", bufs=2
                )
                nc.gpsimd.partition_broadcast(rb[0:64, :], d1, channels=64)
                rh = tp_pool.tile(
                    [64, c.SQ], F32, name="t_rh", tag="t_rh", bufs=2
                )
                nc.gpsimd.partition_broadcast(rh, d2, channels=64)
                nc.sync.dma_start(out=rb[64:128, :], in_=rh)
                nc.vector.tensor_mul(
                    odst[:, hp, :], r(odst[:, hp, :]), rb
                )

        def out_proj_residual(wT_dram, osrc, g_src, xr, xdst_dram):
            with tc.tile_pool(name="ps_op", bufs=3, space="PSUM") as pso:
                for j in range(c.CH):
                    ps = pso.tile([P, c.SQ], F32, name="ps_op", tag="ps_op")
                    wt = wbtile(c.HH)
                    nc.sync.dma_start(
                        out=wt,
                        in_=wT_dram[:, j * P : (j + 1) * P].rearrange(
                            "(k p) m -> p k m", p=P
                        ),
                    )
                    for hp in range(c.HH):
                        nc.tensor.matmul(
                            ps, wt[:, hp, :], osrc[:, hp, :],
                            start=(hp == 0), stop=(hp == c.HH - 1),
                        )
                    t = tw()
                    nc.vector.tensor_mul(t, ps, g_src(j))
                    t2 = tw()
                    nc.vector.tensor_add(t2, t, xr(j))
                    nc.sync.dma_start(
                        out=xdst_dram[j * P : (j + 1) * P, :], in_=t2.bitcast(F32R)
                    )

        def device_ln_stats(x_src):
            """[P, SQ] broadcast (rstd, mean) tiles; x_src(j) -> f32r AP."""
            rs_b = rsp.tile([P, c.SQ], F32, name="t_rsb", tag="t_rsb")
            m_b = rsp.tile([P, c.SQ], F32, name="t_mb", tag="t_mb")
            with tc.tile_pool(name="ps_st", bufs=1, space="PSUM") as psst, \
                 tc.tile_pool(name="stats", bufs=1) as stp:
                ps1 = psst.tile([1, c.SQ], F32, name="ps_st1", tag="ps_st1")
                ps2 = psst.tile([1, c.SQ], F32, name="ps_st2", tag="ps_st2")
                for j in range(c.CH):
                    xa = x_src(j)
                    sq = stp.tile([P, c.SQ], F32R, name="t_sq", tag="t_sq",
                                  bufs=2)
                    nc.vector.tensor_mul(sq, r(xa), r(xa))
                    nc.tensor.matmul(
                        ps1, ONE, xa, start=(j == 0), stop=(j == c.CH - 1)
                    )
                    nc.tensor.matmul(
                        ps2, ONE, sq, start=(j == 0), stop=(j == c.CH - 1)
                    )
                m = stp.tile([1, c.SQ], F32, name="s_m", tag="s_m")
                nc.vector.tensor_scalar_mul(m, ps1[0:1, :], 1.0 / c.D)
                e2 = stp.tile([1, c.SQ], F32, name="s_a", tag="s_a")
                nc.vector.tensor_scalar_mul(e2, ps2[0:1, :], 1.0 / c.D)
                msq = stp.tile([1, c.SQ], F32, name="s_b", tag="s_b")
                nc.vector.tensor_mul(msq, m, m)
                var = stp.tile([1, c.SQ], F32, name="s_c", tag="s_c")
                nc.vector.tensor_sub(var, e2, msq)
                sd = stp.tile([1, c.SQ], F32, name="s_d", tag="s_d")
                nc.scalar.activation(sd, var, AF.Sqrt, bias=EPS[0:1, :])
                rs = stp.tile([1, c.SQ], F32, name="s_e", tag="s_e")
                nc.vector.reciprocal(rs, sd)
                nc.gpsimd.partition_broadcast(rs_b, rs, channels=P)
                nc.gpsimd.partition_broadcast(m_b, m, channels=P)
            return rs_b, m_b

        def stream_x(dram, j, cols):
            t = tw()
            nc.sync.dma_start(out=t, in_=r(dram[j * P : (j + 1) * P, cols]))
            return t

        def stream_xr(dram, j):
            t = tw_pool.tile([P, c.SQ], F32R, name="twr", tag="twr", bufs=2)
            nc.sync.dma_start(out=t, in_=dram[j * P : (j + 1) * P, :])
            return t

        # =======================================================
        # Phase 1: self-attention
        # =======================================================
        with tc.tile_pool(name="p1", bufs=1) as p1:
            QHAT = p1.tile([P, c.HH, c.SQ], F32R)
            KHAT = p1.tile([P, c.HH, c.N], F32R)
            VSELF = p1.tile([P, c.KK, c.H * 65], F32R)

            with tc.tile_pool(name="p1a", bufs=1) as p1a:
                XN = p1a.tile([P, c.CH, c.N], F32R)
                CKs_t = p1a.tile([P, c.N], F32)
                nc.sync.dma_start(out=CKs_t, in_=ckts)
                SKs_t = p1a.tile([P, c.N], F32)
                nc.sync.dma_start(out=SKs_t, in_=skts)
                with tc.tile_pool(name="p1ln", bufs=1) as p1ln:
                    LAs = p1ln.tile([P, c.N], F32)
                    _dma_bcast(nc, LAs, la_self, 0, c.N)
                    LBs = p1ln.tile([P, c.N], F32)
                    _dma_bcast(nc, LBs, lb_self, 0, c.N)
                    ada_modulate(
                        0, 1,
                        lambda j, tf: stream_x(
                            xT, j, slice(tf * c.SQ, (tf + 1) * c.SQ)
                        ),
                        c.NF,
                        lambda cols: LAs[:, cols],
                        lambda cols: LBs[:, cols],
                        lambda j, tf: XN[:, j, tf * c.SQ : (tf + 1) * c.SQ],
                    )
                proj_rope(wqkvT, 0, c.SQ, CQ, SQt, QHAT, XN)
                proj_rope(wqkvT, c.D, c.N, CKs_t, SKs_t, KHAT, XN)
                with tc.tile_pool(name="wvp1", bufs=1) as wvp:
                    vproj_self(XN, VSELF, wvp)

            with tc.tile_pool(name="p1b", bufs=1) as p1b, \
                 tc.tile_pool(name="tp1", bufs=1) as tp1:
                MS = p1b.tile([P, c.KK, c.SQ], F32)
                nc.sync.dma_start(
                    out=MS, in_=mself.rearrange("(k p) q -> p k q", p=P)
                )
                OSELF = p1b.tile([P, c.HH, c.SQ], F32R)

                DENS = p1b.tile([2 * c.HH, c.SQ], F32)
                DENSI = p1b.tile([2 * c.HH, c.SQ], F32)
                with tc.tile_pool(name="ps_oacc", bufs=2, space="PSUM") as psoa:
                    for hp in range(c.HH):
                        ps_o1 = psoa.tile(
                            [65, c.SQ], F32, name="ps_o1", tag="ps_o1"
                        )
                        ps_o2 = psoa.tile(
                            [65, c.SQ], F32, name="ps_o2", tag="ps_o2"
                        )
                        attention_hp(
                            hp, KHAT, VSELF, QHAT, MS, c.KK,
                            ps_o1, ps_o2, tp1, True, True,
                        )
                        evict_unnorm(ps_o1, hp, False, OSELF, DENS, tp1)
                        evict_unnorm(ps_o2, hp, True, OSELF, DENS, tp1)
                normalize_batch(OSELF, DENS, DENSI, tp1, c.HH)

                with tc.tile_pool(name="ps_gx", bufs=2, space="PSUM") as psg:
                    out_proj_residual(
                        wselfT, OSELF, lambda j: ada_gate_one(2, j, psg),
                        lambda j: stream_x(xT, j, slice(0, c.SQ)), xc_d,
                    )

        # =======================================================
        # Phase 2: cross-attention (memory quarters, LN folded into proj)
        # =======================================================
        with tc.tile_pool(name="p2", bufs=1) as p2:
            rs_b, m_b = device_ln_stats(lambda j: stream_xr(xc_d, j))
            QC = p2.tile([P, c.HH, c.SQ], F32R)
            with tc.tile_pool(name="p2q", bufs=1) as p2q:
                XNC = p2q.tile([P, c.CH, c.SQ], F32R)
                ada_modulate(
                    3, 4, lambda j, tf: stream_x(xc_d, j, slice(0, c.SQ)), 1,
                    lambda cols: rs_b[:, cols], lambda cols: m_b[:, cols],
                    lambda j, tf: XNC[:, j, :],
                )
                proj_rope(wqT, 0, c.SQ, CQ, SQt, QC, XNC)

            OACC1 = p2.tile([65, c.HH, c.SQ], F32)
            OACC2 = p2.tile([65, c.HH, c.SQ], F32)
            SWK = p2.tile([P, c.HH], F32)
            nc.sync.dma_start(out=SWK, in_=swk)
            WSVb = p2.tile([P, c.H * c.HD], F32)
            _dma_bcast(nc, WSVb, wsumv, 0, c.H * c.HD)
            RSC = p2.tile([P, 2 * c.KK], F32)
            nc.sync.dma_start(out=RSC, in_=rs_cols)
            MRSC = p2.tile([P, 2 * c.KK], F32)
            nc.sync.dma_start(out=MRSC, in_=mrs_cols)

            nq = 2 * c.NF  # memory quarters over the 2N tokens
            for qq in range(nq):
                half = qq // c.NF            # 0: clean, 1: observed
                hq = qq % c.NF               # quarter index within half
                memT = hcT if half == 0 else hoT
                la_m = la_mc if half == 0 else la_mo
                lb_m = lb_mc if half == 0 else lb_mo
                mmask = mhc if half == 0 else mho
                tok0 = hq * c.SQ             # position offset within half
                ctok = slice(tok0, tok0 + c.SQ)

                with tc.tile_pool(name="p2h", bufs=1) as p2h, \
                     tc.tile_pool(name="mstr", bufs=1) as mstr:
                    MEMQ = p2h.tile([P, c.CH, c.SQ], F32R)
                    nc.sync.dma_start(
                        out=MEMQ,
                        in_=memT[:, ctok].rearrange("(k p) n -> p k n", p=P),
                    )
                    KC = p2h.tile([P, c.HH, c.SQ], F32R)
                    VC = p2h.tile([P, c.QKK, c.H * 65], F32R)
                    CKm_t = p2h.tile([P, c.SQ], F32)
                    nc.sync.dma_start(out=CKm_t, in_=cktm[:, ctok])
                    SKm_t = p2h.tile([P, c.SQ], F32)
                    nc.sync.dma_start(out=SKm_t, in_=sktm[:, ctok])
                    LAm = p2h.tile([P, c.SQ], F32)
                    _dma_bcast(nc, LAm, la_m, tok0, c.SQ)
                    LBm = p2h.tile([P, c.SQ], F32)
                    _dma_bcast(nc, LBm, lb_m, tok0, c.SQ)

                    # ---- K projection: 8 psum banks, stream raw memory ----
                    with tc.tile_pool(name="ps_kp", bufs=1, space="PSUM") as pkp:
                        pks = [
                            pkp.tile([P, c.SQ], F32, name=f"ps_k{h}",
                                     tag=f"ps_k{h}")
                            for h in range(c.HH)
                        ]
                        for hh in range(c.HH):
                            wth = wbtile(c.CH)
                            nc.sync.dma_start(
                                out=wth,
                                in_=wkvT[:, hh * P : (hh + 1) * P].rearrange(
                                    "(k p) m -> p k m", p=P
                                ),
                            )
                            for k in range(c.CH):
                                nc.tensor.matmul(
                                    pks[hh], wth[:, k, :], MEMQ[:, k, :],
                                    start=(k == 0), stop=(k == c.CH - 1),
                                )
                        for hh in range(c.HH):
                            # LN fold: z = ps*rs_t - (mu*rs)_t * rowsum(Wk)
                            t2 = tw()
                            nc.vector.tensor_scalar_mul(
                                t2, LBm, SWK[:, hh : hh + 1]
                            )
                            t1 = tw()
                            nc.vector.tensor_mul(t1, pks[hh], LAm)
                            z = tw()
                            nc.vector.tensor_sub(z, t1, t2)
                            rope_evict(
                                z, hh, slice(0, c.SQ), CKm_t, SKm_t, KC
                            )

                    # ---- V projection (token-major quarter) ----
                    ffw = min(512, c.H * c.HD)
                    nff = (c.H * c.HD) // ffw
                    hpf = ffw // 64
                    for tt in range(c.QKK):
                        ap = VC[:, tt, :].rearrange(
                            "p (h e) -> p h e", e=65
                        )[:, :, 64:65]
                        nc.vector.tensor_copy(ap, ONES16[:, 0 : c.H])
                    with tc.tile_pool(name="ps_v2", bufs=4, space="PSUM") as psv:
                        for ff in range(nff):
                            pss = [
                                psv.tile([P, ffw], F32, name="ps_v2",
                                         tag="ps_v2")
                                for _ in range(c.QKK)
                            ]
                            kh = max(1, c.CH // 4)
                            for kg in range(c.CH // kh):
                                wt = mstr.tile([P, kh, ffw], F32R, name="wv",
                                               tag="wv", bufs=2)
                                nc.sync.dma_start(
                                    out=wt,
                                    in_=wkvT[
                                        kg * kh * P : (kg + 1) * kh * P,
                                        c.D + ff * ffw : c.D + (ff + 1) * ffw,
                                    ].rearrange("(k p) m -> p k m", p=P),
                                )
                                for k in range(kh):
                                    gk = kg * kh + k
                                    for tt in range(c.QKK):
                                        nc.tensor.matmul(
                                            pss[tt],
                                            MEMQ[:, gk, tt * P : (tt + 1) * P],
                                            wt[:, k, :],
                                            start=(gk == 0),
                                            stop=(gk == c.CH - 1),
                                        )
                            for tt in range(c.QKK):
                                tok_col = half * c.KK + hq * c.QKK + tt
                                t2 = mstr.tile(
                                    [P, ffw], F32, name="tvw", tag="tvw",
                                    bufs=2,
                                )
                                nc.vector.tensor_scalar_mul(
                                    t2, WSVb[:, ff * ffw : (ff + 1) * ffw],
                                    MRSC[:, tok_col : tok_col + 1],
                                )
                                ap = VC[
                                    :, tt, ff * hpf * 65 : (ff + 1) * hpf * 65
                                ].rearrange("p (h e) -> p h e", e=65)[:, :, 0:64]
                                nc.vector.scalar_tensor_tensor(
                                    out=ap, in0=pss[tt],
                                    scalar=RSC[:, tok_col : tok_col + 1],
                                    in1=t2, op0=OP.mult, op1=OP.subtract,
                                )

                    # ---- attention over this quarter ----
                    with tc.tile_pool(name="p2ha", bufs=1) as p2ha, \
                         tc.tile_pool(name="tp2", bufs=1) as tp2:
                        MKq = p2ha.tile([P, c.QKK, c.SQ], F32)
                        nc.sync.dma_start(
                            out=MKq,
                            in_=mmask[tok0 : tok0 + c.SQ, :].rearrange(
                                "(k p) q -> p k q", p=P
                            ),
                        )
                        with tc.tile_pool(
                            name="ps_oc", bufs=2, space="PSUM"
                        ) as psoc:
                            for hp in range(c.HH):
                                ps_o1 = psoc.tile(
                                    [65, c.SQ], F32, name="ps_oc1",
                                    tag="ps_oc1",
                                )
                                ps_o2 = psoc.tile(
                                    [65, c.SQ], F32, name="ps_oc2",
                                    tag="ps_oc2",
                                )
                                attention_hp(
                                    hp, KC, VC, QC, MKq, c.QKK,
                                    ps_o1, ps_o2, tp2, True, True,
                                )
                                if qq == 0:
                                    nc.vector.tensor_copy(
                                        OACC1[:, hp, :], ps_o1
                                    )
                                    nc.vector.tensor_copy(
                                        OACC2[:, hp, :], ps_o2
                                    )
                                else:
                                    nc.vector.tensor_add(
                                        OACC1[:, hp, :], OACC1[:, hp, :],
                                        ps_o1,
                                    )
                                    nc.vector.tensor_add(
                                        OACC2[:, hp, :], OACC2[:, hp, :],
                                        ps_o2,
                                    )

            with tc.tile_pool(name="p2n", bufs=1) as p2n:
                OC = p2n.tile([P, c.HH, c.SQ], F32R)
                DENC = p2n.tile([2 * c.HH, c.SQ], F32)
                DENCI = p2n.tile([2 * c.HH, c.SQ], F32)
                with tc.tile_pool(name="tp2n", bufs=1) as tp2n:
                    for hp in range(c.HH):
                        nc.sync.dma_start(
                            out=DENC[2 * hp : 2 * hp + 1, :],
                            in_=OACC1[64:65, hp, :],
                        )
                        nc.sync.dma_start(
                            out=DENC[2 * hp + 1 : 2 * hp + 2, :],
                            in_=OACC2[64:65, hp, :],
                        )
                    nc.vector.reciprocal(DENCI, DENC)
                    for hp in range(c.HH):
                        d1 = small.tile([1, c.SQ], F32, name="s_d1",
                                        tag="s_d1", bufs=2)
                        nc.sync.dma_start(
                            out=d1, in_=DENCI[2 * hp : 2 * hp + 1, :]
                        )
                        d2 = small.tile([1, c.SQ], F32, name="s_d2",
                                        tag="s_d2", bufs=2)
                        nc.sync.dma_start(
                            out=d2, in_=DENCI[2 * hp + 1 : 2 * hp + 2, :]
                        )
                        rb = tp2n.tile(
                            [64, c.SQ], F32, name="t_rb", tag="t_rb", bufs=2
                        )
                        nc.gpsimd.partition_broadcast(rb, d1, channels=64)
                        nc.vector.tensor_mul(
                            OC[0:64, hp, :], OACC1[0:64, hp, :], rb
                        )
                        rh = tp2n.tile(
                            [64, c.SQ], F32, name="t_rh", tag="t_rh", bufs=2
                        )
                        nc.gpsimd.partition_broadcast(rh, d2, channels=64)
                        st = tp2n.tile(
                            [64, c.SQ], F32R, name="t_onorm", tag="t_onorm",
                            bufs=2,
                        )
                        nc.vector.tensor_mul(st, OACC2[0:64, hp, :], rh)
                        nc.sync.dma_start(out=OC[64:128, hp, :], in_=st)
                with tc.tile_pool(name="ps_gx", bufs=2, space="PSUM") as psg:
                    out_proj_residual(
                        wcrossT, OC, lambda j: ada_gate_one(5, j, psg),
                        lambda j: stream_x(xc_d, j, slice(0, c.SQ)), xc2_d,
                    )

        # =======================================================
        # Phase 3: MLP (two hidden halves, SBUF accumulation)
        # =======================================================
        with tc.tile_pool(name="p3", bufs=1) as p3:
            rs_b, m_b = device_ln_stats(lambda j: stream_xr(xc2_d, j))
            OUT_ACC = p3.tile([P, c.CH, c.SQ], F32)

            with tc.tile_pool(name="p3x", bufs=1) as p3x:
                XNM = p3x.tile([P, c.CH, c.SQ], F32R)
                ada_modulate(
                    6, 7, lambda j, tf: stream_x(xc2_d, j, slice(0, c.SQ)), 1,
                    lambda cols: rs_b[:, cols], lambda cols: m_b[:, cols],
                    lambda j, tf: XNM[:, j, :],
                )
                nhalf = c.DHC // 2
                for half in range(2):
                    with tc.tile_pool(name="p3h", bufs=1) as p3h:
                        HT = p3h.tile([P, nhalf, c.SQ], F32R)
                        with tc.tile_pool(
                            name="ps_m1", bufs=3, space="PSUM"
                        ) as psm:
                            for jj in range(nhalf):
                                gj = half * nhalf + jj
                                ps = psm.tile(
                                    [P, c.SQ], F32, name="ps_m1", tag="ps_m1"
                                )
                                wt = wbtile(c.CH)
                                nc.sync.dma_start(
                                    out=wt,
                                    in_=wm1T[
                                        :, gj * P : (gj + 1) * P
                                    ].rearrange("(k p) m -> p k m", p=P),
                                )
                                for k in range(c.CH):
                                    nc.tensor.matmul(
                                        ps, wt[:, k, :], XNM[:, k, :],
                                        start=(k == 0), stop=(k == c.CH - 1),
                                    )
                                nc.scalar.activation(
                                    HT[:, jj, :], ps, AF.Gelu_apprx_tanh,
                                    bias=BM1[:, gj : gj + 1],
                                )
                        with tc.tile_pool(
                            name="ps_m2", bufs=3, space="PSUM"
                        ) as psm2:
                            for j in range(c.CH):
                                ps = psm2.tile(
                                    [P, c.SQ], F32, name="ps_m2", tag="ps_m2"
                                )
                                wt = p3h.tile(
                                    [P, nhalf, P], F32R, name="wm2b",
                                    tag="wm2b", bufs=2,
                                )
                                nc.sync.dma_start(
                                    out=wt,
                                    in_=wm2T[
                                        half * nhalf * P : (half + 1) * nhalf * P,
                                        j * P : (j + 1) * P,
                                    ].rearrange("(k p) m -> p k m", p=P),
                                )
                                for kk_ in range(nhalf):
                                    nc.tensor.matmul(
                                        ps, wt[:, kk_, :], HT[:, kk_, :],
                                        start=(kk_ == 0),
                                        stop=(kk_ == nhalf - 1),
                                    )
                                if half == 0:
                                    nc.vector.tensor_copy(OUT_ACC[:, j, :], ps)
                                else:
                                    nc.vector.tensor_add(
                                        OUT_ACC[:, j, :], OUT_ACC[:, j, :], ps
                                    )

            with tc.tile_pool(name="p3o", bufs=1) as p3o, \
                 tc.tile_pool(name="ps_gx", bufs=2, space="PSUM") as psg:
                OUT = p3o.tile([P, c.CH, c.SQ], F32)
                for j in range(c.CH):
                    gj = ada_gate_one(8, j, psg)
                    t = tw()
                    nc.vector.scalar_tensor_tensor(
                        out=t, in0=OUT_ACC[:, j, :], scalar=BM2[:, j : j + 1],
                        in1=gj, op0=OP.add, op1=OP.mult,
                    )
                    xrj = stream_x(xc2_d, j, slice(0, c.SQ))
                    nc.vector.tensor_add(OUT[:, j, :], t, xrj)
                nc.sync.dma_start(
                    out=out_d.rearrange("(k p) q -> p k q", p=P), in_=OUT
                )

    nc.compile()
    return nc


# =======================================================
# Host side
# =======================================================

def host_prep(cfg: Cfg, inputs: dict):
    c = cfg
    f32 = np.float32

    q_x = np.asarray(inputs["q_x"], f32)
    h_content = np.asarray(inputs["h_content"], f32)
    h_obs = np.asarray(inputs["h_obs"], f32)
    t_cond = np.asarray(inputs["t_cond"], f32)
    M_QQ = np.asarray(inputs["M_QQ"], f32)
    M_hyb = np.asarray(inputs["M_hyb"], f32)
    w_ln_self = np.asarray(inputs["w_ln_self"], f32)
    w_qkv = np.asarray(inputs["w_qkv"], f32)
    w_self_out = np.asarray(inputs["w_self_out"], f32)
    w_ln_cross = np.asarray(inputs["w_ln_cross"], f32)
    w_ln_mem = np.asarray(inputs["w_ln_mem"], f32)
    w_qproj = np.asarray(inputs["w_qproj"], f32)
    w_kvproj = np.asarray(inputs["w_kvproj"], f32)
    w_cross_out = np.asarray(inputs["w_cross_out"], f32)
    w_ln_mlp = np.asarray(inputs["w_ln_mlp"], f32)
    w_mlp1 = np.asarray(inputs["w_mlp1"], f32)
    b_mlp1 = np.asarray(inputs["b_mlp1"], f32)
    w_mlp2 = np.asarray(inputs["w_mlp2"], f32)
    b_mlp2 = np.asarray(inputs["b_mlp2"], f32)
    w_ada = np.asarray(inputs["w_ada"], f32)
    b_ada = np.asarray(inputs["b_ada"], f32)

    D, N, HD, SQ = c.D, c.N, c.HD, c.SQ

    wada9 = w_ada[: 9 * D].copy()
    bada9 = b_ada[: 9 * D].copy()
    for q, wl in ((1, w_ln_self), (4, w_ln_cross), (7, w_ln_mlp)):
        wada9[q * D : (q + 1) * D] *= wl[:, None]
        bada9[q * D : (q + 1) * D] = wl * (1.0 + b_ada[q * D : (q + 1) * D])
    wadaT = np.ascontiguousarray(wada9.T)
    bada_h = np.ascontiguousarray(bada9.reshape(9 * c.CH, P).T)

    wqkvT = np.ascontiguousarray(w_qkv.T)
    wselfT = np.ascontiguousarray(w_self_out.T)
    wqT = np.ascontiguousarray(w_qproj.T)
    wkv_eff = w_kvproj * w_ln_mem[None, :]
    wkvT = np.ascontiguousarray(wkv_eff.T)
    wcrossT = np.ascontiguousarray(w_cross_out.T)
    wm1T = np.ascontiguousarray(w_mlp1.T)
    wm2T = np.ascontiguousarray(w_mlp2.T)
    bm1_h = np.ascontiguousarray(b_mlp1.reshape(c.DHC, P).T)
    bm2_h = np.ascontiguousarray(b_mlp2.reshape(c.CH, P).T)

    # rowsums for the folded memory layernorm
    wsum = wkv_eff.sum(1).astype(f32)          # [2D]
    swk_h = np.ascontiguousarray(wsum[:D].reshape(c.HH, P).T)
    wsumv_h = np.ascontiguousarray(wsum[D:][None, :])

    pos = np.arange(N, dtype=f32)
    inv = (10000.0 ** (-np.arange(0, HD, 2, dtype=f32) / HD)).astype(f32)
    freqs = pos[:, None] * inv[None, :]
    cos64 = np.concatenate([np.cos(freqs), np.cos(freqs)], 1)
    s_sgn = np.concatenate([-np.sin(freqs), np.sin(freqs)], 1)
    c_pair = np.ascontiguousarray(np.tile(cos64.T, (2, 1)).astype(f32))
    s_pair = np.ascontiguousarray(np.tile(s_sgn.T, (2, 1)).astype(f32))
    scale = f32(1.0 / np.sqrt(HD))

    in_maps = []
    for b in range(c.B):
        xb = q_x[b]
        mu_x = xb.mean(-1).astype(f32)
        rs_x = (1.0 / np.sqrt(xb.var(-1) + c.eps)).astype(f32)
        mem = np.concatenate([h_content[b], h_obs[b]], 0)
        mu_m = mem.mean(-1).astype(f32)
        rs_m = (1.0 / np.sqrt(mem.var(-1) + c.eps)).astype(f32)
        mrs_m = (mu_m * rs_m).astype(f32)
        rs_cols_h = np.ascontiguousarray(rs_m.reshape(2 * c.KK, P).T)
        mrs_cols_h = np.ascontiguousarray(mrs_m.reshape(2 * c.KK, P).T)
        mTQQ = np.exp(np.minimum(M_QQ[b].T, 0.0)).astype(f32)
        mThyb = np.exp(np.minimum(M_hyb[b].T, 0.0)).astype(f32)

        for s in range(2):
            own = np.arange(s * SQ, (s + 1) * SQ)
            rest = np.concatenate(
                [np.arange(0, s * SQ), np.arange((s + 1) * SQ, N)]
            )
            perm = np.concatenate([own, rest]).astype(np.int64)
            im = {
                "xT": np.ascontiguousarray(xb.T[:, perm]),
                "tcT": np.ascontiguousarray(t_cond[b].T[:, perm]),
                "hcT": np.ascontiguousarray(h_content[b].T),
                "hoT": np.ascontiguousarray(h_obs[b].T),
                "wadaT": wadaT, "wqkvT": wqkvT, "wselfT": wselfT,
                "wqT": wqT, "wkvT": wkvT, "wcrossT": wcrossT,
                "wm1T": wm1T, "wm2T": wm2T,
                "bada": bada_h, "bm1": bm1_h, "bm2": bm2_h,
                "cqt": np.ascontiguousarray(c_pair[:, perm[:SQ]] * scale),
                "sqt": np.ascontiguousarray(s_pair[:, perm[:SQ]] * scale),
                "ckts": np.ascontiguousarray(c_pair[:, perm]),
                "skts": np.ascontiguousarray(s_pair[:, perm]),
                "cktm": c_pair, "sktm": s_pair,
                "mself": np.ascontiguousarray(mTQQ[perm][:, perm[:SQ]]),
                "mhc": np.ascontiguousarray(mThyb[:N][:, perm[:SQ]]),
                "mho": np.ascontiguousarray(mThyb[N:][:, perm[:SQ]]),
                "la_self": np.ascontiguousarray(rs_x[perm][None, :]),
                "lb_self": np.ascontiguousarray(mu_x[perm][None, :]),
                "la_mc": np.ascontiguousarray(rs_m[:N][None, :]),
                "lb_mc": np.ascontiguousarray(mrs_m[:N][None, :]),
                "la_mo": np.ascontiguousarray(rs_m[N:][None, :]),
                "lb_mo": np.ascontiguousarray(mrs_m[N:][None, :]),
                "swk": swk_h, "wsumv": wsumv_h,
                "rs_cols": rs_cols_h, "mrs_cols": mrs_cols_h,
            }
            in_maps.append(im)
    return in_maps


_PROGRAM_CACHE = {}


def get_program(cfg: Cfg):
    key = (cfg.N, cfg.D, cfg.H)
    if key not in _PROGRAM_CACHE:
        _PROGRAM_CACHE[key] = build_program(cfg)
    return _PROGRAM_CACHE[key]


def assemble(cfg: Cfg, results):
    c = cfg
    out = np.zeros((c.B, c.N, c.D), np.float32)
    for b in range(c.B):
        for s in range(2):
            o = results[2 * b + s]["out"]
            out[b, s * c.SQ : (s + 1) * c.SQ, :] = o.T
    return out


def kernel(**inputs) -> np.ndarray:
    cfg = Cfg(mini=False)
    nc = get_program(cfg)
    in_maps = host_prep(cfg, inputs)
    res = bass_utils.run_bass_kernel_spmd(
        nc, in_maps, core_ids=list(range(cfg.n_cores)), trace=False
    )
    return assemble(cfg, res.results)



# revision 9
# speedup vs baseline: 1.4367x; 1.4367x over previous
"""Trainium2 Bass kernel for a DiT-style transformer block (adaLN modulation,
RoPE self-attention with additive rank mask, hybrid cross-attention to
[clean|observed] memory, gated MLP).

Sharding: 8 cores = 4 batches x 2 sequence-halves; each core computes the
block output for its 512 query tokens of one batch.

Key structure (v2):
- All matmul operands in bf16 (PE full rate at any free size, half DMA/SBUF).
- Rank-sorted token order: the additive masks are rank comparisons, so
  sorting queries/keys by rank makes the masked region of every 128-key
  chunk a contiguous query prefix/suffix. Host derives per-chunk static
  free-dim bounds from the actual masks (min/max across cores) and the
  program skips fully-masked score/exp/mask/pv columns.
- Probabilities in bf16; masking is a single DVE bf16 multiply (2x mode).
- Softmax denominators from a ones-column in V (row 64 of each head block);
  normalization via per-head reciprocal + gpsimd row broadcast.
- Memory layernorm applied on device (2 bf16 DVE ops per chunk) before the
  shared KV projection; cross-attention K/V for all 2N memory tokens are
  SBUF-resident so attention accumulates straight in PSUM across 16 chunks.
- MLP hidden (bf16) fully SBUF-resident; second matmul accumulates all 32
  k-chunks in one PSUM group.
- Residual stream kept in fp32 SBUF across phases (no DRAM scratch).
"""

import numpy as np
from contextlib import ExitStack

from concourse import bacc, mybir
import concourse.bass as bass
import concourse.tile as tile
from concourse import bass_utils

F32 = mybir.dt.float32
F32R = mybir.dt.float32r
BF16 = mybir.dt.bfloat16
AF = mybir.ActivationFunctionType
OP = mybir.AluOpType

P = 128


class Cfg:
    def __init__(self, mini=False):
        self.B, self.N, self.D, self.H, self.HD = 4, 1024, 1024, 16, 64
        self.COND = 256
        self.DH = 4 * self.D
        self.SQ = self.N // 2            # own query tokens per core
        self.CH = self.D // P            # d-chunks (8)
        self.HH = self.H * self.HD // P  # head-pair chunks (8)
        self.KK = self.N // P            # key chunks per N tokens (8)
        self.NF = self.N // self.SQ      # token-free blocks of SQ (2)
        self.CC = self.COND // P         # cond chunks (2)
        self.DHC = self.DH // P          # mlp hidden chunks (32)
        self.n_cores = 2 * self.B
        self.eps = 1e-5


def _dma_bcast(nc, out_tile, dram_ap, off, n):
    """DMA dram row [1, off:off+n] broadcast to all partitions [parts, n]."""
    parts = out_tile.shape[0]
    src = bass.AP(
        tensor=dram_ap.tensor, offset=dram_ap.offset + off, ap=[[0, parts], [1, n]]
    )
    nc.gpsimd.dma_start(out=out_tile, in_=src)


def r(ap):
    """fp32 view of an f32r AP for DVE/ACT input reads."""
    return ap.bitcast(F32)


def build_program(cfg: Cfg, bounds):
    c = cfg
    qs_self, qs_clean, qe_obs = bounds
    nc = bacc.Bacc(
        "TRN2",
        target_bir_lowering=False,
        debug=False,
        enable_asserts=True,
        num_devices=c.n_cores,
    )

    def din(name, shape, dt=BF16):
        return nc.dram_tensor(name, shape, dt, kind="ExternalInput").ap()

    xT = din("xT", [c.D, c.N], F32R)
    tcT = din("tcT", [c.COND, c.N])
    hcT = din("hcT", [c.D, c.N])
    hoT = din("hoT", [c.D, c.N])
    wadaT = din("wadaT", [c.COND, 9 * c.D])
    wqkvT = din("wqkvT", [c.D, 3 * c.D])
    wselfT = din("wselfT", [c.D, c.D])
    wqT = din("wqT", [c.D, c.D])
    wkvT = din("wkvT", [c.D, 2 * c.D])
    wcrossT = din("wcrossT", [c.D, c.D])
    wm1T = din("wm1T", [c.D, c.DH])
    wm2T = din("wm2T", [c.DH, c.D])
    bada = din("bada", [P, 9 * c.CH], F32)
    bm1 = din("bm1", [P, c.DHC], F32)
    bm2 = din("bm2", [P, c.CH], F32)
    cqt = din("cqt", [P, c.SQ])
    sqt = din("sqt", [P, c.SQ])
    ckts = din("ckts", [P, c.N])
    skts = din("skts", [P, c.N])
    cktm = din("cktm", [P, c.N])
    sktm = din("sktm", [P, c.N])
    mself = din("mself", [c.N, c.SQ])
    mhc = din("mhc", [c.N, c.SQ])
    mho = din("mho", [c.N, c.SQ])
    la_self = din("la_self", [1, c.N], F32)   # rstd per own-order token
    lb_self = din("lb_self", [1, c.N], F32)   # mean per own-order token
    la_mem = din("la_mem", [1, 2 * c.N])      # rstd per sorted [clean|obs] tok
    lb_mem = din("lb_mem", [1, 2 * c.N])      # mean*rstd per sorted mem tok
    out_d = nc.dram_tensor("out", [c.D, c.SQ], F32, kind="ExternalOutput").ap()

    with ExitStack() as ctx:
        tc = ctx.enter_context(tile.TileContext(nc))
        persist = ctx.enter_context(tc.tile_pool(name="persist", bufs=1))
        ws = ctx.enter_context(tc.tile_pool(name="wstream", bufs=1))
        tw_pool = ctx.enter_context(tc.tile_pool(name="tw", bufs=5))
        rsp = ctx.enter_context(tc.tile_pool(name="rsp", bufs=1))
        small = ctx.enter_context(tc.tile_pool(name="small", bufs=1))

        def wbtile(nk):
            return ws.tile([P, nk, P], BF16, name=f"wb{nk}", tag=f"wb{nk}",
                           bufs=4)

        def tw():
            return tw_pool.tile([P, c.SQ], F32, name="tw", tag="tw")

        def twb():
            return tw_pool.tile([P, c.SQ], BF16, name="twb", tag="twb", bufs=4)

        # ---------- persistent preloads ----------
        TC = persist.tile([P, c.CC, c.N], BF16)
        nc.sync.dma_start(out=TC, in_=tcT.rearrange("(k p) n -> p k n", p=P))
        BADA = persist.tile([P, 9 * c.CH], F32)
        nc.sync.dma_start(out=BADA, in_=bada)
        BM1 = persist.tile([P, c.DHC], F32)
        nc.sync.dma_start(out=BM1, in_=bm1)
        BM2 = persist.tile([P, c.CH], F32)
        nc.sync.dma_start(out=BM2, in_=bm2)

        EPS = persist.tile([P, 1], F32)
        nc.vector.memset(EPS, 1e-5)
        ones_f32 = persist.tile([P, 16], F32)
        nc.vector.memset(ones_f32, 1.0)
        ONE = persist.tile([P, 1], F32R)
        nc.vector.tensor_copy(ONE, ones_f32[:, 0:1])
        ONESB = persist.tile([P, 16], BF16)
        nc.vector.tensor_copy(ONESB, ones_f32)

        # residual stream, fp32 SBUF-resident across phases
        XC = persist.tile([P, c.CH, c.SQ], F32R)
        XC2 = persist.tile([P, c.CH, c.SQ], F32R)

        shift_eng = [0]

        def shift32(dst, src):
            """dst[p] = src[p xor-32 within each 64-block]; SBUF tiles."""
            eng = nc.sync if shift_eng[0] == 0 else nc.gpsimd
            shift_eng[0] ^= 1
            for blk in range(2):
                b = blk * 64
                eng.dma_start(out=dst[b : b + 32, :], in_=src[b + 32 : b + 64, :])
                eng.dma_start(out=dst[b + 32 : b + 64, :], in_=src[b : b + 32, :])

        # ---------- helpers ----------
        def ada_modulate(q_sh, q_sc, x_src, x_nf, la_b, lb_b, xn_dst):
            """xn = (x - m)*sc1 + sh; sc1 = la*(ps_sc + bada_sc);
            sh = ps_sh + bada_sh. la_b/lb_b(cols) give [P, SQ] f32 APs."""
            with tc.tile_pool(name="ps_ada", bufs=1, space="PSUM") as psa:
                for j in range(c.CH):
                    ps_sh = [
                        psa.tile([P, c.SQ], F32, name=f"ps_sh{t}", tag=f"ps_sh{t}")
                        for t in range(x_nf)
                    ]
                    ps_sc = [
                        psa.tile([P, c.SQ], F32, name=f"ps_sc{t}", tag=f"ps_sc{t}")
                        for t in range(x_nf)
                    ]
                    wt = wbtile(c.CC)
                    nc.sync.dma_start(
                        out=wt,
                        in_=wadaT[
                            :, q_sh * c.D + j * P : q_sh * c.D + (j + 1) * P
                        ].rearrange("(k p) m -> p k m", p=P),
                    )
                    wt2 = wbtile(c.CC)
                    nc.sync.dma_start(
                        out=wt2,
                        in_=wadaT[
                            :, q_sc * c.D + j * P : q_sc * c.D + (j + 1) * P
                        ].rearrange("(k p) m -> p k m", p=P),
                    )
                    for k in range(c.CC):
                        for tf in range(x_nf):
                            nc.tensor.matmul(
                                ps_sh[tf], wt[:, k, :],
                                TC[:, k, tf * c.SQ : (tf + 1) * c.SQ],
                                start=(k == 0), stop=(k == c.CC - 1),
                            )
                        for tf in range(x_nf):
                            nc.tensor.matmul(
                                ps_sc[tf], wt2[:, k, :],
                                TC[:, k, tf * c.SQ : (tf + 1) * c.SQ],
                                start=(k == 0), stop=(k == c.CC - 1),
                            )
                    for tf in range(x_nf):
                        cols = slice(tf * c.SQ, (tf + 1) * c.SQ)
                        sc1 = tw()
                        nc.vector.scalar_tensor_tensor(
                            out=sc1, in0=ps_sc[tf],
                            scalar=BADA[:, q_sc * c.CH + j : q_sc * c.CH + j + 1],
                            in1=la_b(cols), op0=OP.add, op1=OP.mult,
                        )
                        xm = tw()
                        nc.vector.tensor_sub(xm, x_src(j, tf), lb_b(cols))
                        t = tw()
                        nc.vector.tensor_mul(t, xm, sc1)
                        nc.vector.scalar_tensor_tensor(
                            out=xn_dst(j, tf), in0=ps_sh[tf],
                            scalar=BADA[:, q_sh * c.CH + j : q_sh * c.CH + j + 1],
                            in1=t, op0=OP.add, op1=OP.add,
                        )

        def ada_gate_one(q_g, j, psg):
            """Return a [P, SQ] f32 tile holding gate chunk j on demand."""
            ps = psg.tile([P, c.SQ], F32, name="ps_g", tag="ps_g")
            wt = wbtile(c.CC)
            nc.sync.dma_start(
                out=wt,
                in_=wadaT[
                    :, q_g * c.D + j * P : q_g * c.D + (j + 1) * P
                ].rearrange("(k p) m -> p k m", p=P),
            )
            for k in range(c.CC):
                nc.tensor.matmul(
                    ps, wt[:, k, :], TC[:, k, 0 : c.SQ],
                    start=(k == 0), stop=(k == c.CC - 1),
                )
            g = tw()
            nc.vector.tensor_scalar_add(
                g, ps, BADA[:, q_g * c.CH + j : q_g * c.CH + j + 1]
            )
            return g

        def rope_evict(ps, hh, cols, ctab, stab, dst, tp):
            """dst[:, hh, cols] = z*cos + shift32(z)*sin_signed, z=copy(ps)."""
            n = ctab.shape[-1]
            z = tp.tile([P, c.SQ], BF16, name="t_z", tag="t_z", bufs=3)
            nc.scalar.activation(z[:, 0:n], ps, AF.Copy)
            t1 = tp.tile([P, c.SQ], BF16, name="t_r1", tag="t_r1", bufs=3)
            nc.vector.tensor_mul(t1[:, 0:n], z[:, 0:n], ctab)
            zs = tp.tile([P, c.SQ], BF16, name="t_zs", tag="t_zs", bufs=3)
            shift32(zs[:, 0:n], z[:, 0:n])
            nc.vector.tensor_mul(zs[:, 0:n], zs[:, 0:n], stab)
            nc.vector.tensor_add(dst[:, hh, cols], t1[:, 0:n], zs[:, 0:n])

        def proj_rope(wT_dram, col_off, n_free, ctab, stab, dst, src, tp):
            """dst[:, hh, :] = rope(W[:, cols].T @ src), head-pair chunks.

            src(k) -> [P, n_free] bf16 AP (moving operand)."""
            nf = n_free // c.SQ
            with tc.tile_pool(name="ps_qk", bufs=4, space="PSUM") as psq:
                for hh in range(c.HH):
                    wt = wbtile(c.CH)
                    nc.sync.dma_start(
                        out=wt,
                        in_=wT_dram[
                            :, col_off + hh * P : col_off + (hh + 1) * P
                        ].rearrange("(k p) m -> p k m", p=P),
                    )
                    for tf in range(nf):
                        ps = psq.tile([P, c.SQ], F32, name="ps_qk", tag="ps_qk")
                        cols = slice(tf * c.SQ, (tf + 1) * c.SQ)
                        for k in range(c.CH):
                            nc.tensor.matmul(
                                ps, wt[:, k, :], src(k)[:, cols],
                                start=(k == 0), stop=(k == c.CH - 1),
                            )
                        rope_evict(
                            ps, hh, cols, ctab[:, cols], stab[:, cols], dst, tp
                        )

        def vproj(w_dram, w_col0, src, ntt, vdst, wvp, tok0=0):
            """vdst[:, tok0+tt, :] = (src.T @ Wv) per 128-token tile, with a
            ones column appended per head. src(k) -> [P, ntt*P] bf16 AP."""
            ffw = 512
            nff = (c.H * c.HD) // ffw
            hpf = ffw // c.HD
            for tt in range(ntt):
                ap = vdst[:, tok0 + tt, :].rearrange(
                    "p (h e) -> p h e", e=c.HD + 1
                )[:, :, c.HD : c.HD + 1]
                nc.vector.tensor_copy(ap, ONESB[:, 0 : c.H])
            with tc.tile_pool(name="ps_v", bufs=8, space="PSUM") as psv:
                for ff in range(nff):
                    pss = [
                        psv.tile([P, ffw], F32, name="ps_v", tag="ps_v")
                        for _ in range(ntt)
                    ]
                    kh = 2
                    for kg in range(c.CH // kh):
                        wt = wvp.tile([P, kh, ffw], BF16, name="wv", tag="wv",
                                      bufs=3)
                        nc.sync.dma_start(
                            out=wt,
                            in_=w_dram[
                                kg * kh * P : (kg + 1) * kh * P,
                                w_col0 + ff * ffw : w_col0 + (ff + 1) * ffw,
                            ].rearrange("(k p) m -> p k m", p=P),
                        )
                        for k in range(kh):
                            gk = kg * kh + k
                            for tt in range(ntt):
                                nc.tensor.matmul(
                                    pss[tt],
                                    src(gk)[:, tt * P : (tt + 1) * P],
                                    wt[:, k, :],
                                    start=(gk == 0), stop=(gk == c.CH - 1),
                                )
                    for tt in range(ntt):
                        ap = (
                            vdst[:, tok0 + tt,
                                 ff * hpf * (c.HD + 1) : (ff + 1) * hpf * (c.HD + 1)]
                            .rearrange("p (h e) -> p h e", e=c.HD + 1)[:, :, 0:c.HD]
                        )
                        nc.vector.tensor_copy(ap, pss[tt])

        def attention_hp(hp, khat, vtile, qhat, chunks, ps_o1, ps_o2, tp):
            """One head pair over a static chunk list.

            chunks: list of (kidx, c0, c1, mask_ap_fn); first entry must span
            [0, SQ). pv lags scores by one chunk for pipelining."""
            h1, h2 = 2 * hp, 2 * hp + 1
            nchunks = len(chunks)

            def pv(kidx, pt, c0, c1, first, last):
                nc.tensor.matmul(
                    ps_o1[:, c0:c1],
                    vtile[:, kidx, h1 * 65 : (h1 + 1) * 65],
                    pt[:, 0, c0:c1],
                    start=first, stop=last, skip_group_check=True,
                )
                nc.tensor.matmul(
                    ps_o2[:, c0:c1],
                    vtile[:, kidx, h2 * 65 : (h2 + 1) * 65],
                    pt[:, 1, c0:c1],
                    start=first, stop=last, skip_group_check=True,
                )

            with tc.tile_pool(name="ps_s", bufs=2, space="PSUM") as pss:
                prev = None
                for i, (kidx, c0, c1, mfn) in enumerate(chunks):
                    ps = pss.tile([P, 2, c.SQ], F32, name="ps_s", tag="ps_s")
                    ks = slice(kidx * P, (kidx + 1) * P)
                    nc.tensor.matmul(
                        ps[:, 0, c0:c1],
                        khat[0:64, hp, ks], qhat[0:64, hp, c0:c1],
                        start=True, stop=True,
                    )
                    nc.tensor.matmul(
                        ps[:, 1, c0:c1],
                        khat[64:128, hp, ks], qhat[64:128, hp, c0:c1],
                        start=True, stop=True,
                    )
                    pt = tp.tile([P, 2, c.SQ], BF16, name="t_p", tag="t_p",
                                 bufs=3)
                    nc.scalar.activation(pt[:, 0, c0:c1], ps[:, 0, c0:c1],
                                         AF.Exp)
                    nc.scalar.activation(pt[:, 1, c0:c1], ps[:, 1, c0:c1],
                                         AF.Exp)
                    m = mfn()
                    nc.vector.tensor_mul(pt[:, 0, c0:c1], pt[:, 0, c0:c1],
                                         m[:, c0:c1])
                    nc.vector.tensor_mul(pt[:, 1, c0:c1], pt[:, 1, c0:c1],
                                         m[:, c0:c1])
                    if prev is not None:
                        pv(*prev)
                    prev = (kidx, pt, c0, c1, i == 0, i == nchunks - 1)
                pv(*prev)

        def attention_all(khat, vtile, qhat, chunks, odst, tp):
            """All head pairs; normalize each into odst [P, HH, SQ] bf16."""
            with tc.tile_pool(name="ps_oacc", bufs=2, space="PSUM") as psoa:
                for hp in range(c.HH):
                    ps_o1 = psoa.tile([65, c.SQ], F32, name="ps_o1",
                                      tag="ps_o1")
                    ps_o2 = psoa.tile([65, c.SQ], F32, name="ps_o2",
                                      tag="ps_o2")
                    attention_hp(hp, khat, vtile, qhat, chunks, ps_o1, ps_o2,
                                 tp)
                    # normalize: odst rows 0:64 = o1/den1, 64:128 = o2/den2
                    r1 = small.tile([1, c.SQ], F32, name="s_r1", tag="s_r1",
                                    bufs=2)
                    nc.vector.reciprocal(r1, ps_o1[64:65, :])
                    r2 = small.tile([1, c.SQ], F32, name="s_r2", tag="s_r2",
                                    bufs=2)
                    nc.vector.reciprocal(r2, ps_o2[64:65, :])
                    rb1 = tp.tile([64, c.SQ], F32, name="t_rb1", tag="t_rb1",
                                  bufs=2)
                    nc.gpsimd.partition_broadcast(rb1, r1, channels=64)
                    rb2 = tp.tile([64, c.SQ], F32, name="t_rb2", tag="t_rb2",
                                  bufs=2)
                    nc.gpsimd.partition_broadcast(rb2, r2, channels=64)
                    nc.vector.tensor_mul(odst[0:64, hp, :], ps_o1[0:64, :],
                                         rb1)
                    o2 = tp.tile([64, c.SQ], BF16, name="t_o2", tag="t_o2",
                                 bufs=2)
                    nc.vector.tensor_mul(o2, ps_o2[0:64, :], rb2)
                    nc.sync.dma_start(out=odst[64:128, hp, :], in_=o2)

        def out_proj_residual(wT_dram, osrc, g_q, xr, xdst):
            """xdst[:, j, :] = xr(j) + gate_j * (W.T @ osrc)"""
            with tc.tile_pool(name="ps_op", bufs=3, space="PSUM") as pso, \
                 tc.tile_pool(name="ps_gx", bufs=2, space="PSUM") as psg:
                for j in range(c.CH):
                    ps = pso.tile([P, c.SQ], F32, name="ps_op", tag="ps_op")
                    wt = wbtile(c.HH)
                    nc.sync.dma_start(
                        out=wt,
                        in_=wT_dram[:, j * P : (j + 1) * P].rearrange(
                            "(k p) m -> p k m", p=P
                        ),
                    )
                    for hp in range(c.HH):
                        nc.tensor.matmul(
                            ps, wt[:, hp, :], osrc[:, hp, :],
                            start=(hp == 0), stop=(hp == c.HH - 1),
                        )
                    g = ada_gate_one(g_q, j, psg)
                    t = tw()
                    nc.vector.tensor_mul(t, ps, g)
                    nc.vector.tensor_add(xdst[:, j, :], t, xr(j))

        def device_ln_stats(x_src):
            """[P, SQ] broadcast (rstd, mean) f32 tiles; x_src(j) -> f32r."""
            rs_b = rsp.tile([P, c.SQ], F32, name="t_rsb", tag="t_rsb")
            m_b = rsp.tile([P, c.SQ], F32, name="t_mb", tag="t_mb")
            with tc.tile_pool(name="ps_st", bufs=1, space="PSUM") as psst, \
                 tc.tile_pool(name="stats", bufs=1) as stp:
                ps1 = psst.tile([1, c.SQ], F32, name="ps_st1", tag="ps_st1")
                ps2 = psst.tile([1, c.SQ], F32, name="ps_st2", tag="ps_st2")
                for j in range(c.CH):
                    xa = x_src(j)
                    sq = stp.tile([P, c.SQ], F32R, name="t_sq", tag="t_sq",
                                  bufs=2)
                    nc.vector.tensor_mul(sq, r(xa), r(xa))
                    nc.tensor.matmul(
                        ps1, ONE, xa, start=(j == 0), stop=(j == c.CH - 1)
                    )
                    nc.tensor.matmul(
                        ps2, ONE, sq, start=(j == 0), stop=(j == c.CH - 1)
                    )
                m = stp.tile([1, c.SQ], F32, name="s_m", tag="s_m")
                nc.vector.tensor_scalar_mul(m, ps1[0:1, :], 1.0 / c.D)
                e2 = stp.tile([1, c.SQ], F32, name="s_a", tag="s_a")
                nc.vector.tensor_scalar_mul(e2, ps2[0:1, :], 1.0 / c.D)
                msq = stp.tile([1, c.SQ], F32, name="s_b", tag="s_b")
                nc.vector.tensor_mul(msq, m, m)
                var = stp.tile([1, c.SQ], F32, name="s_c", tag="s_c")
                nc.vector.tensor_sub(var, e2, msq)
                sd = stp.tile([1, c.SQ], F32, name="s_d", tag="s_d")
                nc.scalar.activation(sd, var, AF.Sqrt, bias=EPS[0:1, :])
                rs = stp.tile([1, c.SQ], F32, name="s_e", tag="s_e")
                nc.vector.reciprocal(rs, sd)
                nc.gpsimd.partition_broadcast(rs_b, rs, channels=P)
                nc.gpsimd.partition_broadcast(m_b, m, channels=P)
            return rs_b, m_b

        def stream_x(dram, j, cols):
            t = tw()
            nc.sync.dma_start(out=t, in_=r(dram[j * P : (j + 1) * P, cols]))
            return t

        # =======================================================
        # Phase 1: self-attention
        # =======================================================
        with tc.tile_pool(name="p1", bufs=1) as p1:
            QHAT = p1.tile([P, c.HH, c.SQ], BF16)
            KHAT = p1.tile([P, c.HH, c.N], BF16)
            VSELF = p1.tile([P, c.KK, c.H * 65], BF16)

            with tc.tile_pool(name="p1a", bufs=1) as p1a, \
                 tc.tile_pool(name="tp0", bufs=1) as tp0:
                XN = p1a.tile([P, c.CH, c.N], BF16)
                CQ = p1a.tile([P, c.SQ], BF16)
                nc.sync.dma_start(out=CQ, in_=cqt)
                SQt = p1a.tile([P, c.SQ], BF16)
                nc.sync.dma_start(out=SQt, in_=sqt)
                CKs_t = p1a.tile([P, c.N], BF16)
                nc.sync.dma_start(out=CKs_t, in_=ckts)
                SKs_t = p1a.tile([P, c.N], BF16)
                nc.sync.dma_start(out=SKs_t, in_=skts)
                with tc.tile_pool(name="p1ln", bufs=1) as p1ln:
                    LAs = p1ln.tile([P, c.N], F32)
                    _dma_bcast(nc, LAs, la_self, 0, c.N)
                    LBs = p1ln.tile([P, c.N], F32)
                    _dma_bcast(nc, LBs, lb_self, 0, c.N)
                    ada_modulate(
                        0, 1,
                        lambda j, tf: stream_x(
                            xT, j, slice(tf * c.SQ, (tf + 1) * c.SQ)
                        ),
                        c.NF,
                        lambda cols: LAs[:, cols],
                        lambda cols: LBs[:, cols],
                        lambda j, tf: XN[:, j, tf * c.SQ : (tf + 1) * c.SQ],
                    )
                proj_rope(wqkvT, 0, c.SQ, CQ, SQt, QHAT,
                          lambda k: XN[:, k, :], tp0)
                proj_rope(wqkvT, c.D, c.N, CKs_t, SKs_t, KHAT,
                          lambda k: XN[:, k, :], tp0)
                with tc.tile_pool(name="wvp1", bufs=1) as wvp:
                    vproj(wqkvT, 2 * c.D, lambda k: XN[:, k, :], c.KK,
                          VSELF, wvp)

            with tc.tile_pool(name="p1b", bufs=1) as p1b, \
                 tc.tile_pool(name="tp1", bufs=1) as tp1:
                MS = p1b.tile([P, c.KK, c.SQ], BF16)
                nc.sync.dma_start(
                    out=MS, in_=mself.rearrange("(k p) q -> p k q", p=P)
                )
                OSELF = p1b.tile([P, c.HH, c.SQ], BF16)
                chunks = []
                for kkc in range(c.KK):
                    q0 = qs_self[kkc]
                    if q0 >= c.SQ:
                        continue
                    chunks.append(
                        (kkc, q0, c.SQ,
                         (lambda kk_: lambda: MS[:, kk_, :])(kkc))
                    )
                attention_all(KHAT, VSELF, QHAT, chunks, OSELF, tp1)
                out_proj_residual(
                    wselfT, OSELF, 2,
                    lambda j: stream_x(xT, j, slice(0, c.SQ)), XC,
                )

        # =======================================================
        # Phase 2: cross-attention to [clean | observed] memory
        # =======================================================
        with tc.tile_pool(name="p2", bufs=1) as p2:
            rs_b, m_b = device_ln_stats(lambda j: XC[:, j, :])
            QC = p2.tile([P, c.HH, c.SQ], BF16)
            KMEM = p2.tile([P, c.HH, 2 * c.N], BF16)
            VMEM = p2.tile([P, 2 * c.KK, c.H * 65], BF16)

            with tc.tile_pool(name="p2q", bufs=1) as p2q, \
                 tc.tile_pool(name="tp2q", bufs=1) as tp2q:
                XNC = p2q.tile([P, c.CH, c.SQ], BF16)
                CQ2 = p2q.tile([P, c.SQ], BF16)
                nc.sync.dma_start(out=CQ2, in_=cqt)
                SQ2 = p2q.tile([P, c.SQ], BF16)
                nc.sync.dma_start(out=SQ2, in_=sqt)
                ada_modulate(
                    3, 4, lambda j, tf: r(XC[:, j, :]), 1,
                    lambda cols: rs_b[:, cols], lambda cols: m_b[:, cols],
                    lambda j, tf: XNC[:, j, :],
                )
                proj_rope(wqT, 0, c.SQ, CQ2, SQ2, QC,
                          lambda k: XNC[:, k, :], tp2q)

            # ---- project K/V for all 2N memory tokens (rank-sorted) ----
            for qq in range(4):
                half = qq // 2            # 0: clean, 1: observed
                hq = qq % 2               # half-index within source tensor
                memT = hcT if half == 0 else hoT
                tok0 = hq * c.SQ
                ctok = slice(tok0, tok0 + c.SQ)
                moff = half * c.N + tok0  # offset into sorted memory stats

                with tc.tile_pool(name="p2h", bufs=2) as p2h, \
                     tc.tile_pool(name="p2n", bufs=1) as p2n, \
                     tc.tile_pool(name="tp2h", bufs=1) as tp2h:
                    MEMQ = p2h.tile([P, c.CH, c.SQ], BF16, name="memq",
                                    tag="memq")
                    nc.sync.dma_start(
                        out=MEMQ,
                        in_=memT[:, ctok].rearrange("(k p) n -> p k n", p=P),
                    )
                    RSm = p2h.tile([P, c.SQ], BF16, name="rsm", tag="rsm")
                    _dma_bcast(nc, RSm, la_mem, moff, c.SQ)
                    MRSm = p2h.tile([P, c.SQ], BF16, name="mrsm", tag="mrsm")
                    _dma_bcast(nc, MRSm, lb_mem, moff, c.SQ)
                    CKm = p2h.tile([P, c.SQ], BF16, name="ckm", tag="ckm")
                    nc.sync.dma_start(out=CKm, in_=cktm[:, ctok])
                    SKm = p2h.tile([P, c.SQ], BF16, name="skm", tag="skm")
                    nc.sync.dma_start(out=SKm, in_=sktm[:, ctok])

                    MEMN = p2n.tile([P, c.CH, c.SQ], BF16, name="memn",
                                    tag="memn")
                    for j in range(c.CH):
                        t = twb()
                        nc.vector.tensor_mul(t, MEMQ[:, j, :], RSm)
                        nc.vector.tensor_sub(MEMN[:, j, :], t, MRSm)

                    # K projection for this quarter
                    with tc.tile_pool(name="ps_kp", bufs=4, space="PSUM") \
                            as pkp:
                        for hh in range(c.HH):
                            wth = wbtile(c.CH)
                            nc.sync.dma_start(
                                out=wth,
                                in_=wkvT[:, hh * P : (hh + 1) * P].rearrange(
                                    "(k p) m -> p k m", p=P
                                ),
                            )
                            ps = pkp.tile([P, c.SQ], F32, name="ps_kp",
                                          tag="ps_kp")
                            for k in range(c.CH):
                                nc.tensor.matmul(
                                    ps, wth[:, k, :], MEMN[:, k, :],
                                    start=(k == 0), stop=(k == c.CH - 1),
                                )
                            rope_evict(
                                ps, hh,
                                slice(qq * c.SQ, (qq + 1) * c.SQ),
                                CKm, SKm, KMEM, tp2h,
                            )

                    # V projection for this quarter
                    with tc.tile_pool(name="wvp2", bufs=1) as wvp2:
                        vproj(wkvT, c.D, lambda k: MEMN[:, k, :], 4,
                              VMEM, wvp2, tok0=qq * 4)

            # ---- attention over all 16 memory chunks ----
            with tc.tile_pool(name="p2b", bufs=1) as p2b, \
                 tc.tile_pool(name="tp2", bufs=1) as tp2:
                MHC = p2b.tile([P, c.KK, c.SQ], BF16)
                nc.sync.dma_start(
                    out=MHC, in_=mhc.rearrange("(k p) q -> p k q", p=P)
                )
                MHO = p2b.tile([P, c.KK, c.SQ], BF16)
                nc.sync.dma_start(
                    out=MHO, in_=mho.rearrange("(k p) q -> p k q", p=P)
                )
                OC = p2b.tile([P, c.HH, c.SQ], BF16)
                chunks = []
                for j in range(c.KK - 1, -1, -1):   # observed, descending
                    qe = qe_obs[j]
                    if qe <= 0:
                        continue
                    chunks.append(
                        (c.KK + j, 0, qe,
                         (lambda jj: lambda: MHO[:, jj, :])(j))
                    )
                for cc in range(c.KK):              # clean, ascending
                    q0 = qs_clean[cc]
                    if q0 >= c.SQ:
                        continue
                    chunks.append(
                        (cc, q0, c.SQ,
                         (lambda jj: lambda: MHC[:, jj, :])(cc))
                    )
                attention_all(KMEM, VMEM, QC, chunks, OC, tp2)
                out_proj_residual(
                    wcrossT, OC, 5, lambda j: r(XC[:, j, :]), XC2,
                )

        # =======================================================
        # Phase 3: MLP
        # =======================================================
        with tc.tile_pool(name="p3", bufs=1) as p3:
            rs_b, m_b = device_ln_stats(lambda j: XC2[:, j, :])

            XNM = p3.tile([P, c.CH, c.SQ], BF16)
            ada_modulate(
                6, 7, lambda j, tf: r(XC2[:, j, :]), 1,
                lambda cols: rs_b[:, cols], lambda cols: m_b[:, cols],
                lambda j, tf: XNM[:, j, :],
            )
            HT = p3.tile([P, c.DHC, c.SQ], BF16)
            with tc.tile_pool(name="ps_m1", bufs=4, space="PSUM") as psm:
                for jj in range(c.DHC):
                    ps = psm.tile([P, c.SQ], F32, name="ps_m1", tag="ps_m1")
                    wt = wbtile(c.CH)
                    nc.sync.dma_start(
                        out=wt,
                        in_=wm1T[:, jj * P : (jj + 1) * P].rearrange(
                            "(k p) m -> p k m", p=P
                        ),
                    )
                    for k in range(c.CH):
                        nc.tensor.matmul(
                            ps, wt[:, k, :], XNM[:, k, :],
                            start=(k == 0), stop=(k == c.CH - 1),
                        )
                    nc.scalar.activation(
                        HT[:, jj, :], ps, AF.Gelu_apprx_tanh,
                        bias=BM1[:, jj : jj + 1],
                    )

            with tc.tile_pool(name="p3o", bufs=1) as p3o, \
                 tc.tile_pool(name="ps_m2", bufs=3, space="PSUM") as psm2, \
                 tc.tile_pool(name="ps_gx", bufs=2, space="PSUM") as psg:
                OUT = p3o.tile([P, c.CH, c.SQ], F32)
                for j in range(c.CH):
                    ps = psm2.tile([P, c.SQ], F32, name="ps_m2", tag="ps_m2")
                    wt = p3o.tile([P, c.DHC, P], BF16, name="wm2b",
                                  tag="wm2b", bufs=2)
                    nc.sync.dma_start(
                        out=wt,
                        in_=wm2T[:, j * P : (j + 1) * P].rearrange(
                            "(k p) m -> p k m", p=P
                        ),
                    )
                    for kk_ in range(c.DHC):
                        nc.tensor.matmul(
                            ps, wt[:, kk_, :], HT[:, kk_, :],
                            start=(kk_ == 0), stop=(kk_ == c.DHC - 1),
                        )
                    g = ada_gate_one(8, j, psg)
                    t = tw()
                    nc.vector.scalar_tensor_tensor(
                        out=t, in0=ps, scalar=BM2[:, j : j + 1],
                        in1=g, op0=OP.add, op1=OP.mult,
                    )
                    nc.vector.tensor_add(OUT[:, j, :], t, r(XC2[:, j, :]))
                nc.sync.dma_start(
                    out=out_d.rearrange("(k p) q -> p k q", p=P), in_=OUT
                )

    nc.compile()
    return nc


# =======================================================
# Host side
# =======================================================

def host_prep(cfg: Cfg, inputs: dict):
    c = cfg
    f32 = np.float32
    import ml_dtypes
    bf16 = ml_dtypes.bfloat16

    q_x = np.asarray(inputs["q_x"], f32)
    h_content = np.asarray(inputs["h_content"], f32)
    h_obs = np.asarray(inputs["h_obs"], f32)
    t_cond = np.asarray(inputs["t_cond"], f32)
    M_QQ = np.asarray(inputs["M_QQ"], f32)
    M_hyb = np.asarray(inputs["M_hyb"], f32)
    w_ln_self = np.asarray(inputs["w_ln_self"], f32)
    w_qkv = np.asarray(inputs["w_qkv"], f32)
    w_self_out = np.asarray(inputs["w_self_out"], f32)
    w_ln_cross = np.asarray(inputs["w_ln_cross"], f32)
    w_ln_mem = np.asarray(inputs["w_ln_mem"], f32)
    w_qproj = np.asarray(inputs["w_qproj"], f32)
    w_kvproj = np.asarray(inputs["w_kvproj"], f32)
    w_cross_out = np.asarray(inputs["w_cross_out"], f32)
    w_ln_mlp = np.asarray(inputs["w_ln_mlp"], f32)
    w_mlp1 = np.asarray(inputs["w_mlp1"], f32)
    b_mlp1 = np.asarray(inputs["b_mlp1"], f32)
    w_mlp2 = np.asarray(inputs["w_mlp2"], f32)
    b_mlp2 = np.asarray(inputs["b_mlp2"], f32)
    w_ada = np.asarray(inputs["w_ada"], f32)
    b_ada = np.asarray(inputs["b_ada"], f32)

    D, N, HD, SQ = c.D, c.N, c.HD, c.SQ

    wada9 = w_ada[: 9 * D].copy()
    bada9 = b_ada[: 9 * D].copy()
    for q, wl in ((1, w_ln_self), (4, w_ln_cross), (7, w_ln_mlp)):
        wada9[q * D : (q + 1) * D] *= wl[:, None]
        bada9[q * D : (q + 1) * D] = wl * (1.0 + b_ada[q * D : (q + 1) * D])
    wadaT = np.ascontiguousarray(wada9.T.astype(bf16))
    bada_h = np.ascontiguousarray(bada9.reshape(9 * c.CH, P).T)

    wqkvT = np.ascontiguousarray(w_qkv.T.astype(bf16))
    wselfT = np.ascontiguousarray(w_self_out.T.astype(bf16))
    wqT = np.ascontiguousarray(w_qproj.T.astype(bf16))
    wkv_eff = w_kvproj * w_ln_mem[None, :]
    wkvT = np.ascontiguousarray(wkv_eff.T.astype(bf16))
    wcrossT = np.ascontiguousarray(w_cross_out.T.astype(bf16))
    wm1T = np.ascontiguousarray(w_mlp1.T.astype(bf16))
    wm2T = np.ascontiguousarray(w_mlp2.T.astype(bf16))
    bm1_h = np.ascontiguousarray(b_mlp1.reshape(c.DHC, P).T)
    bm2_h = np.ascontiguousarray(b_mlp2.reshape(c.CH, P).T)

    pos = np.arange(N, dtype=f32)
    inv = (10000.0 ** (-np.arange(0, HD, 2, dtype=f32) / HD)).astype(f32)
    freqs = pos[:, None] * inv[None, :]
    cos64 = np.concatenate([np.cos(freqs), np.cos(freqs)], 1)
    s_sgn = np.concatenate([-np.sin(freqs), np.sin(freqs)], 1)
    c_pair = np.ascontiguousarray(np.tile(cos64.T, (2, 1)).astype(f32))
    s_pair = np.ascontiguousarray(np.tile(s_sgn.T, (2, 1)).astype(f32))
    scale = f32(1.0 / np.sqrt(HD))

    in_maps = []
    perms = []
    qs_self_all, qs_clean_all, qe_obs_all = [], [], []
    for b in range(c.B):
        xb = q_x[b]
        mu_x = xb.mean(-1).astype(f32)
        rs_x = (1.0 / np.sqrt(xb.var(-1) + c.eps)).astype(f32)
        mem = np.concatenate([h_content[b], h_obs[b]], 0)
        mu_m = mem.mean(-1).astype(f32)
        rs_m = (1.0 / np.sqrt(mem.var(-1) + c.eps)).astype(f32)
        mrs_m = (mu_m * rs_m).astype(f32)
        allow_qq = M_QQ[b] == 0.0            # [q, k]
        allow_c = M_hyb[b, :, :N] == 0.0     # [q, mem]
        allow_o = M_hyb[b, :, N:] == 0.0

        # rank order: allowed-count is monotone in rank
        qcnt = allow_qq.sum(1)
        mem_perm = np.argsort(-allow_c.sum(0), kind="stable").astype(np.int64)

        for s in range(2):
            own_idx = np.arange(s * SQ, (s + 1) * SQ)
            rest_idx = np.concatenate(
                [np.arange(0, s * SQ), np.arange((s + 1) * SQ, N)]
            )
            own = own_idx[np.argsort(qcnt[own_idx], kind="stable")]
            rest = rest_idx[np.argsort(qcnt[rest_idx], kind="stable")]
            perm = np.concatenate([own, rest]).astype(np.int64)
            perms.append(perm)

            mS = allow_qq.T[perm][:, perm[:SQ]]          # [key, q] bool
            mC = allow_c.T[mem_perm][:, perm[:SQ]]
            mO = allow_o.T[mem_perm][:, perm[:SQ]]

            def first_alive(mask_rows):
                nz = np.flatnonzero(mask_rows.any(0))
                return int(nz[0]) if nz.size else SQ

            def last_alive(mask_rows):
                nz = np.flatnonzero(mask_rows.any(0))
                return int(nz[-1] + 1) if nz.size else 0

            qs_self_all.append([
                first_alive(mS[kc * P : (kc + 1) * P]) for kc in range(c.KK)
            ])
            qs_clean_all.append([
                first_alive(mC[kc * P : (kc + 1) * P]) for kc in range(c.KK)
            ])
            qe_obs_all.append([
                last_alive(mO[kc * P : (kc + 1) * P]) for kc in range(c.KK)
            ])

            im = {
                "xT": np.ascontiguousarray(xb.T[:, perm]),
                "tcT": np.ascontiguousarray(t_cond[b].T[:, perm].astype(bf16)),
                "hcT": np.ascontiguousarray(
                    h_content[b].T[:, mem_perm].astype(bf16)),
                "hoT": np.ascontiguousarray(
                    h_obs[b].T[:, mem_perm].astype(bf16)),
                "wadaT": wadaT, "wqkvT": wqkvT, "wselfT": wselfT,
                "wqT": wqT, "wkvT": wkvT, "wcrossT": wcrossT,
                "wm1T": wm1T, "wm2T": wm2T,
                "bada": bada_h, "bm1": bm1_h, "bm2": bm2_h,
                "cqt": np.ascontiguousarray(
                    (c_pair[:, perm[:SQ]] * scale).astype(bf16)),
                "sqt": np.ascontiguousarray(
                    (s_pair[:, perm[:SQ]] * scale).astype(bf16)),
                "ckts": np.ascontiguousarray(c_pair[:, perm].astype(bf16)),
                "skts": np.ascontiguousarray(s_pair[:, perm].astype(bf16)),
                "cktm": np.ascontiguousarray(c_pair[:, mem_perm].astype(bf16)),
                "sktm": np.ascontiguousarray(s_pair[:, mem_perm].astype(bf16)),
                "mself": np.ascontiguousarray(mS.astype(bf16)),
                "mhc": np.ascontiguousarray(mC.astype(bf16)),
                "mho": np.ascontiguousarray(mO.astype(bf16)),
                "la_self": np.ascontiguousarray(rs_x[perm][None, :]),
                "lb_self": np.ascontiguousarray(mu_x[perm][None, :]),
                "la_mem": np.ascontiguousarray(np.concatenate(
                    [rs_m[:N][mem_perm], rs_m[N:][mem_perm]]
                )[None, :].astype(bf16)),
                "lb_mem": np.ascontiguousarray(np.concatenate(
                    [mrs_m[:N][mem_perm], mrs_m[N:][mem_perm]]
                )[None, :].astype(bf16)),
            }
            in_maps.append(im)

    def reduce_bounds(lists, is_start):
        arr = np.array(lists)              # [cores, KK]
        if is_start:
            v = (arr.min(0) // 32) * 32
        else:
            v = np.minimum(((arr.max(0) + 31) // 32) * 32, SQ)
        return [int(x) for x in v]

    qs_self = reduce_bounds(qs_self_all, True)
    qs_self[0] = 0                 # first processed chunk must span all cols
    qs_clean = reduce_bounds(qs_clean_all, True)
    qe_obs = reduce_bounds(qe_obs_all, False)
    qe_obs[c.KK - 1] = SQ          # first processed chunk must span all cols
    bounds = (tuple(qs_self), tuple(qs_clean), tuple(qe_obs))
    return in_maps, perms, bounds


_PROGRAM_CACHE = {}
NEEDS_INPUTS = True


def get_program(cfg: Cfg, bounds):
    if bounds not in _PROGRAM_CACHE:
        _PROGRAM_CACHE[bounds] = build_program(cfg, bounds)
    return _PROGRAM_CACHE[bounds]


def assemble(cfg: Cfg, results, perms):
    c = cfg
    out = np.zeros((c.B, c.N, c.D), np.float32)
    for b in range(c.B):
        for s in range(2):
            core = 2 * b + s
            o = results[core]["out"]
            out[b, perms[core][: c.SQ], :] = o.T
    return out


def kernel(**inputs) -> np.ndarray:
    cfg = Cfg(mini=False)
    in_maps, perms, bounds = host_prep(cfg, inputs)
    nc = get_program(cfg, bounds)
    res = bass_utils.run_bass_kernel_spmd(
        nc, in_maps, core_ids=list(range(cfg.n_cores)), trace=False
    )
    return assemble(cfg, res.results, perms)
